# revision 25
# baseline (speedup 1.0000x reference)
"""PaPs loss kernel for Trainium2 (8 NeuronCores, SPMD data-parallel).

Sharding: core c handles batch c's center-loss image and the 256 centers
k in [256c, 256c+256) (size/class/shape losses). Each core emits partial
sums; the host combines them in float64.

Fast path (inputs matching the canonical 16x16 stride-12 center grid):
the per-center 64x64 crop reads are served by 16 static-access-pattern
HWDGE DMAs that load, for partition p = 8a+b, the union block
rows [12a,12a+64) x cols [24b,24b+76) covering both of its crops; the
crop-vs-zone compare is then 2 large strided DVE ops. instance_masks
travel as bf16. A generic indirect-DMA path remains for any other
center/box layout.
"""

import numpy as np

B, H, W, S, GRID, NCLS = 8, 256, 256, 64, 16, 20
K = B * GRID * GRID            # 2048 centers total
KC = K // 8                    # 256 centers per core
EPS = 1e-8
P = 128
PIX = H * W                    # 65536
UW = S + 12                    # union block width (two crops 12 apart)

TRACE = False
LAST_EXEC_NS = None
LAST_PROFILE = None
_CACHE = {}


SP_BETA = 0.306
SP_ALPHA = 1.0 / SP_BETA
SP_GAMMA = 0.69344386


def _build_fast(bf_ok):
    from concourse import bass, bacc, mybir
    import concourse.tile as tile

    f32 = mybir.dt.float32
    bf16 = mybir.dt.bfloat16
    fp8 = mybir.dt.float8e4
    udt = bf16 if bf_ok else f32
    Alu = mybir.AluOpType
    Act = mybir.ActivationFunctionType
    AxX = mybir.AxisListType.X
    HS = S // 2

    nc = bacc.Bacc()
    hm_d = nc.declare_dram_parameter("hm", [P, 512], f32, isOutput=False)
    g0_d = nc.declare_dram_parameter("g0", [P, 512], bf16, isOutput=False)
    msk_d = nc.declare_dram_parameter("msk", [P, 2 * S, S], fp8, isOutput=False)
    sem_d = nc.declare_dram_parameter("sem", [P, 2, NCLS], f32, isOutput=False)
    szp_d = nc.declare_dram_parameter("szp", [P, 2, 2], f32, isOutput=False)
    tgt_d = nc.declare_dram_parameter("tgt", [1, PIX, 7], f32, isOutput=False)
    pan_d = nc.declare_dram_parameter("pan", [8, H, UW], udt, isOutput=False)
    out_d = nc.declare_dram_parameter("out", [16], f32, isOutput=True)

    with tile.TileContext(nc) as tc:
        with (
            tc.tile_pool(name="sb", bufs=1) as sb,
            tc.tile_pool(name="ps", bufs=1, space="PSUM") as ps,
        ):
            acc = sb.tile([P, 16], f32)
            nc.vector.memset(acc[:], 0.0)
            ceps = sb.tile([P, 1], f32)
            nc.vector.memset(ceps[:], EPS)
            c1eps = sb.tile([P, 1], f32)
            nc.vector.memset(c1eps[:], 1.0 + EPS)
            cbeta = sb.tile([P, 1], f32)
            nc.vector.memset(cbeta[:], -SP_BETA)

            # --- sync queue DMAs; crop-compare inputs (msk, sem, cg,
            # unions) first so the stt stream starts early; Act-tail inputs
            # (g0, hm) after
            msk = sb.tile([P, 2 * S, S], fp8)
            nc.sync.dma_start(out=msk[:, 0:S, :], in_=msk_d[:, 0:S, :])
            nc.sync.dma_start(out=msk[:, S:2 * S, :], in_=msk_d[:, S:2 * S, :])
            sem = sb.tile([P, 2, NCLS], f32)
            nc.sync.dma_start(out=sem[:], in_=sem_d[:])
            # per-center [zone, size0, size1, label] = target channels 2..5
            # at center pixel (12a, 24b+12j) for center k = 16a+2b+j = 2p+j
            cg = sb.tile([P, 2, 4], f32)
            for j in range(2):
                src = bass.AP(tensor=tgt_d, offset=2 + 12 * j * 7,
                              ap=[[3072 * 7, 16], [24 * 7, 8], [1, 4]])
                nc.sync.dma_start(out=cg[:, j, :], in_=src)
            # union blocks: partition p = 8a+b gets rows [12a,12a+64) of
            # column panel b (cols [24b,24b+76) of the instance image);
            # panel rows are contiguous -> two 128-descriptor DMAs
            union = sb.tile([P, S, UW], udt)
            for h in range(2):
                src = bass.AP(tensor=pan_d, offset=HS * UW * h,
                              ap=[[12 * UW, 16], [H * UW, 8], [1, HS * UW]])
                nc.sync.dma_start(out=union[:, HS * h:HS * (h + 1), :], in_=src)
            g0 = sb.tile([P, 512], bf16)
            nc.sync.dma_start(out=g0[:], in_=g0_d[:])
            hm = sb.tile([P, 512], f32)
            nc.sync.dma_start(out=hm[:], in_=hm_d[:])
            szp = sb.tile([P, 2, 2], f32)
            nc.sync.dma_start(out=szp[:], in_=szp_d[:])
            # heatmap at the K centers (the guarded positives): pos_l comes
            # from these 2 values per partition instead of a full-image pass
            hmc = sb.tile([P, 2], f32)
            for j in range(2):
                src = bass.AP(tensor=hm_d, offset=12 * j,
                              ap=[[3072, 16], [24, 8], [1, 1]])
                nc.sync.dma_start(out=hmc[:, j:j + 1], in_=src)

            # --- early DVE: dm half a, the small class/zone ops in the
            # msk_b wait, dm half b between the crop-compare quarters
            dm = sb.tile([P, 2 * S, S], bf16)
            nc.vector.tensor_scalar(out=dm[:, 0:S, :], in0=msk[:, 0:S, :],
                                    scalar1=2.0, scalar2=-1.0,
                                    op0=Alu.mult, op1=Alu.add)
            M = sb.tile([P, 2], f32)
            for j in range(2):
                nc.vector.tensor_reduce(out=M[:, j:j + 1], in_=sem[:, j],
                                        axis=AxX, op=Alu.max)
            negM = sb.tile([P, 2], f32)
            nc.vector.tensor_scalar(out=negM[:], in0=M[:], scalar1=-1.0,
                                    scalar2=None, op0=Alu.mult)
            nc.vector.tensor_reduce(out=acc[:, 12:13], in_=M[:], axis=AxX,
                                    op=Alu.add)
            zb = sb.tile([P, 2], udt)
            nc.vector.tensor_copy(zb[:], cg[:, :, 0])

            # --- Act engine: Gelu table first (softplus pass), then one
            # switch to the ln+exp table; Square taps live in every table
            gel = sb.tile([P, 2 * S, S], bf16)
            nc.scalar.activation(out=gel[:, 0:S, :], in_=msk[:, 0:S, :],
                                 func=Act.Gelu, bias=cbeta[:],
                                 scale=2.0 * SP_BETA, accum_out=acc[:, 3:4])
            nc.scalar.activation(out=gel[:, S:2 * S, :], in_=msk[:, S:2 * S, :],
                                 func=Act.Gelu, bias=cbeta[:],
                                 scale=2.0 * SP_BETA, accum_out=acc[:, 4:5])
            u2 = sb.tile([P, 512], f32)
            nc.scalar.activation(out=u2[:], in_=g0[:], func=Act.Square,
                                 bias=1.0, scale=-1.0)
            u4 = sb.tile([P, 512], f32)
            nc.scalar.activation(out=u4[:], in_=u2[:], func=Act.Square,
                                 bias=0.0, scale=1.0)
            logn = sb.tile([P, 512], f32)
            nc.scalar.activation(out=logn[:], in_=hm[:], func=Act.Ln,
                                 bias=c1eps[:], scale=-1.0)
            sexp = sb.tile([P, 2], f32)
            eb = sb.tile([P, 2, NCLS], f32)
            for j in range(2):
                nc.scalar.activation(out=eb[:, j], in_=sem[:, j], func=Act.Exp,
                                     bias=negM[:, j:j + 1], scale=1.0,
                                     accum_out=sexp[:, j:j + 1])
            lnS = sb.tile([P, 2], f32)
            nc.scalar.activation(out=lnS[:], in_=sexp[:], func=Act.Ln,
                                 bias=0.0, scale=1.0, accum_out=acc[:, 11:12])
            lnhmc = sb.tile([P, 2], f32)
            nc.scalar.activation(out=lnhmc[:], in_=hmc[:], func=Act.Ln,
                                 bias=ceps[:], scale=1.0, accum_out=acc[:, 1:2])

            # --- GpSimd: iota only
            io_i = sb.tile([P, NCLS], mybir.dt.int32)
            nc.gpsimd.iota(io_i[:], pattern=[[1, NCLS]], base=0,
                           channel_multiplier=0)

            nc.vector.tensor_scalar(out=dm[:, S:2 * S, :], in0=msk[:, S:2 * S, :],
                                    scalar1=2.0, scalar2=-1.0,
                                    op0=Alu.mult, op1=Alu.add)

            # --- DVE: crop-compare quarters
            def _stt(j, h):
                tj = sb.tile([P, HS, S], bf16, name=f"tj{j}{h}")
                r0, r1 = HS * h, HS * (h + 1)
                nc.vector.scalar_tensor_tensor(
                    out=tj[:], in0=union[:, r0:r1, 12 * j:12 * j + S],
                    scalar=zb[:, j:j + 1],
                    in1=dm[:, S * j + r0:S * j + r1, :],
                    op0=Alu.is_equal, op1=Alu.mult,
                    accum_out=acc[:, 5 + 2 * j + h:6 + 2 * j + h])

            _stt(0, 0)
            _stt(1, 0)
            _stt(0, 1)
            _stt(1, 1)

            # --- center-loss negative term
            wl = sb.tile([P, 512], f32)
            nc.vector.tensor_tensor(out=wl[:], in0=u4[:], in1=logn[:],
                                    op=Alu.mult)
            t2 = sb.tile([P, 512], f32)
            nc.vector.scalar_tensor_tensor(out=t2[:], in0=g0[:], scalar=1.0,
                                           in1=wl[:], op0=Alu.is_lt,
                                           op1=Alu.mult, accum_out=acc[:, 2:3])

            # --- class loss partials
            io_f = sb.tile([P, NCLS], f32)
            nc.vector.tensor_copy(io_f[:], io_i[:])
            for j in range(2):
                tmp20 = sb.tile([P, NCLS], f32, name=f"tmp20_{j}")
                nc.vector.scalar_tensor_tensor(out=tmp20[:], in0=io_f[:],
                                               scalar=cg[:, j, 3:4],
                                               in1=sem[:, j],
                                               op0=Alu.is_equal, op1=Alu.mult,
                                               accum_out=acc[:, 13 + j:14 + j])

            # --- size loss partials: |true - pred| / (true + eps)
            tsz = cg[:, :, 1:3]
            d = sb.tile([P, 2, 2], f32)
            nc.vector.tensor_tensor(out=d[:], in0=tsz, in1=szp[:],
                                    op=Alu.subtract)
            den = sb.tile([P, 2, 2], f32)
            nc.vector.tensor_scalar(out=den[:], in0=tsz, scalar1=EPS,
                                    scalar2=None, op0=Alu.add)
            rec = sb.tile([P, 2, 2], f32)
            nc.vector.reciprocal(out=rec[:], in_=den[:])
            q = sb.tile([P, 2, 2], f32)
            nc.vector.tensor_tensor(out=q[:], in0=d[:], in1=rec[:],
                                    op=Alu.mult)
            nc.vector.tensor_reduce(out=acc[:, 10:11], in_=q[:],
                                    axis=mybir.AxisListType.XY, op=Alu.add,
                                    apply_absolute_value=True)

            # --- cross-partition reduction of the 16 accumulator columns
            ones = sb.tile([P, 1], f32)
            nc.vector.memset(ones[:], 1.0)
            psum = ps.tile([16, 1], f32, space="PSUM")
            nc.tensor.matmul(out=psum[:], lhsT=acc[:], rhs=ones[:],
                             start=True, stop=True)
            res = sb.tile([16, 1], f32)
            nc.vector.tensor_copy(res[:], psum[:])
            nc.sync.dma_start(out=out_d[:], in_=res[:, 0])

    nc.compile()
    return nc


def _is_grid(inputs):
    centers = np.asarray(inputs["centers_bij"])
    boxes = np.asarray(inputs["boxes"])
    if centers.shape != (K, 3) or boxes.shape != (K, 4):
        return False
    gi, gj = np.meshgrid(np.arange(GRID), np.arange(GRID), indexing='ij')
    ii = np.tile((gi * 12).reshape(-1), B)
    jj = np.tile((gj * 12).reshape(-1), B)
    bb = np.repeat(np.arange(B), GRID * GRID)
    return (np.array_equal(centers[:, 0], bb)
            and np.array_equal(centers[:, 1], ii)
            and np.array_equal(centers[:, 2], jj)
            and np.array_equal(boxes[:, 1], ii)
            and np.array_equal(boxes[:, 0], jj))


def _prepare_fast(inputs, bf_ok):
    import ml_dtypes
    heatmap = np.asarray(inputs["heatmap"], dtype=np.float32)
    size_pred = np.asarray(inputs["size_pred"], dtype=np.float32)
    semantic_pred = np.asarray(inputs["semantic_pred"], dtype=np.float32)
    target = np.asarray(inputs["target"], dtype=np.float32)
    msk_bf = np.asarray(inputs["instance_masks"], dtype=np.float32).astype(
        ml_dtypes.float8_e4m3fn)
    udt = ml_dtypes.bfloat16 if bf_ok else np.float32
    in_maps = []
    for c in range(8):
        sl = slice(KC * c, KC * (c + 1))
        inst = target[c, :, :, 1].astype(udt)
        pan = np.stack([np.ascontiguousarray(inst[:, 24 * b:24 * b + UW])
                        for b in range(8)])
        in_maps.append({
            "hm": np.ascontiguousarray(heatmap[c]).reshape(P, 512),
            "g0": target[c, :, :, 0].astype(ml_dtypes.bfloat16).reshape(P, 512),
            "msk": np.ascontiguousarray(msk_bf[sl]).reshape(P, 2 * S, S),
            "sem": np.ascontiguousarray(semantic_pred[sl]).reshape(P, 2, NCLS),
            "szp": np.ascontiguousarray(size_pred[sl]).reshape(P, 2, 2),
            "tgt": np.ascontiguousarray(target[c]).reshape(1, PIX, 7),
            "pan": pan,
        })
    return in_maps


def _bf16_exact(inputs):
    """True if instance ids / zone ids round-trip exactly through bf16
    (then bf16 equality is equivalent to f32 equality)."""
    import ml_dtypes
    target = np.asarray(inputs["target"], dtype=np.float32)
    vals = target[:, :, :, 1:3]
    return bool(np.all(vals.astype(ml_dtypes.bfloat16).astype(np.float32) == vals))


def _fast_ok(inputs):
    """Extra fast-path requirements: heatmap positives sit exactly at the
    canonical centers (pos_l reduces to a 2048-point gather) and the
    instance masks fit fp8's (0,1) range."""
    target = np.asarray(inputs["target"], dtype=np.float32)
    g = target[:, :, :, 0]
    pos = g == 1.0
    gi, gj = np.meshgrid(np.arange(GRID), np.arange(GRID), indexing='ij')
    ii = np.tile((gi * 12).reshape(-1), B)
    jj = np.tile((gj * 12).reshape(-1), B)
    bb = np.repeat(np.arange(B), GRID * GRID)
    if not (bool(pos[bb, ii, jj].all()) and int(pos.sum()) == K):
        return False
    if not bool(np.all(pos | (g < 0.995))):
        return False
    m = np.asarray(inputs["instance_masks"], dtype=np.float32)
    return bool(m.min() >= 0.0 and m.max() <= 1.0)


# ---------------------------------------------------------------------------
# generic fallback: per-row indirect-DMA gathers, correct for any
# centers_bij/boxes values
# ---------------------------------------------------------------------------

def _build_generic(nb):
    from concourse import bass, bacc, mybir
    import concourse.tile as tile

    f32 = mybir.dt.float32
    i32 = mybir.dt.int32
    Alu = mybir.AluOpType
    Act = mybir.ActivationFunctionType
    AxX = mybir.AxisListType.X

    nc = bacc.Bacc()
    hm_d = nc.declare_dram_parameter("hm", [P, 512], f32, isOutput=False)
    tgt_d = nc.declare_dram_parameter("tgt", [1, nb * PIX, 7], f32, isOutput=False)
    msk_d = nc.declare_dram_parameter("msk", [P, 2 * S, S], f32, isOutput=False)
    sem_d = nc.declare_dram_parameter("sem", [P, 2, NCLS], f32, isOutput=False)
    szp_d = nc.declare_dram_parameter("szp", [P, 2, 2], f32, isOutput=False)
    cidx_d = nc.declare_dram_parameter("cidx", [P, 2], i32, isOutput=False)
    ridx_d = nc.declare_dram_parameter("ridx", [P, 128], i32, isOutput=False)
    inst_d = nc.declare_dram_parameter("inst", [1, nb * PIX + 256, 1], f32,
                                       isOutput=False)
    out_d = nc.declare_dram_parameter("out", [16], f32, isOutput=True)

    with tile.TileContext(nc) as tc:
        with (
            tc.tile_pool(name="sb", bufs=1) as sb,
            tc.tile_pool(name="ps", bufs=1, space="PSUM") as ps,
        ):
            acc = sb.tile([P, 16], f32)
            nc.vector.memset(acc[:], 0.0)

            hm = sb.tile([P, 512], f32)
            nc.sync.dma_start(out=hm[:], in_=hm_d[:])
            msk = sb.tile([P, 2 * S, S], f32)
            nc.sync.dma_start(out=msk[:], in_=msk_d[:])
            sem = sb.tile([P, 2, NCLS], f32)
            nc.sync.dma_start(out=sem[:], in_=sem_d[:])
            szp = sb.tile([P, 2, 2], f32)
            nc.sync.dma_start(out=szp[:], in_=szp_d[:])
            cidx = sb.tile([P, 2], i32)
            nc.sync.dma_start(out=cidx[:], in_=cidx_d[:])
            ridx = sb.tile([P, 128], i32)
            nc.sync.dma_start(out=ridx[:], in_=ridx_d[:])

            # --- batch c's target tile (channel 0 feeds the center loss)
            tsb0 = sb.tile([P, 512, 7], f32)
            nc.sync.dma_start(out=tsb0[:], in_=tgt_d[0:1, 0:PIX])

            # --- per-center gather: [zone, size0, size1, label] (channels 2..5)
            # HW SWDGE honors only ONE index per partition, so one gather per j
            cg = sb.tile([P, 2, 4], f32)
            for j in range(2):
                nc.gpsimd.indirect_dma_start(
                    out=cg[:, j], out_offset=None,
                    in_=tgt_d[:],
                    in_offset=bass.IndirectOffsetOnAxis(ap=cidx[:, j:j + 1],
                                                        axis=1),
                    element_offset=2,
                )

            # --- center loss partials (batch c image, one [128,512] tile)
            g0 = tsb0[:, :, 0]
            ceps = sb.tile([P, 1], f32)
            nc.vector.memset(ceps[:], EPS)
            c1eps = sb.tile([P, 1], f32)
            nc.vector.memset(c1eps[:], 1.0 + EPS)
            logp = sb.tile([P, 512], f32)
            nc.scalar.activation(out=logp[:], in_=hm[:], func=Act.Ln,
                                 bias=ceps[:], scale=1.0)
            logn = sb.tile([P, 512], f32)
            nc.scalar.activation(out=logn[:], in_=hm[:], func=Act.Ln,
                                 bias=c1eps[:], scale=-1.0)
            posm = sb.tile([P, 512], f32)
            nc.vector.tensor_scalar(out=posm[:], in0=g0, scalar1=1.0,
                                    scalar2=0.0, op0=Alu.is_equal,
                                    op1=Alu.add, accum_out=acc[:, 0:1])
            t1 = sb.tile([P, 512], f32)
            nc.vector.scalar_tensor_tensor(out=t1[:], in0=g0, scalar=1.0,
                                           in1=logp[:], op0=Alu.is_equal,
                                           op1=Alu.mult, accum_out=acc[:, 1:2])
            u = sb.tile([P, 512], f32)
            nc.vector.tensor_scalar(out=u[:], in0=g0, scalar1=-1.0,
                                    scalar2=1.0, op0=Alu.mult, op1=Alu.add)
            u2 = sb.tile([P, 512], f32)
            nc.vector.tensor_tensor(out=u2[:], in0=u[:], in1=u[:], op=Alu.mult)
            u4 = sb.tile([P, 512], f32)
            nc.vector.tensor_tensor(out=u4[:], in0=u2[:], in1=u2[:], op=Alu.mult)
            wl = sb.tile([P, 512], f32)
            nc.vector.tensor_tensor(out=wl[:], in0=u4[:], in1=logn[:], op=Alu.mult)
            t2 = sb.tile([P, 512], f32)
            nc.vector.scalar_tensor_tensor(out=t2[:], in0=g0, scalar=1.0,
                                           in1=wl[:], op0=Alu.is_lt,
                                           op1=Alu.mult, accum_out=acc[:, 2:3])

            # --- shape loss partials
            # per-elem loss = softplus(1-2m) + (1-t)*(2m-1); sum decomposes as
            # sum(sp) + sum(dm) - sum(t*dm)
            dm = sb.tile([P, 2 * S, S], f32)
            nc.vector.tensor_scalar(out=dm[:], in0=msk[:], scalar1=2.0,
                                    scalar2=-1.0, op0=Alu.mult, op1=Alu.add)
            nc.vector.tensor_reduce(out=acc[:, 3:4], in_=dm[:],
                                    axis=mybir.AxisListType.XY, op=Alu.add)
            e = sb.tile([P, 2 * S, S], f32)
            nc.scalar.activation(out=e[:], in_=dm[:], func=Act.Exp,
                                 bias=0.0, scale=-1.0)
            nc.scalar.activation(out=e[:], in_=e[:], func=Act.Ln,
                                 bias=1.0, scale=1.0, accum_out=acc[:, 4:5])
            # crop(k) rows arrive as 128 single-index gathers of one 64-px
            # row each (run starts at the crop's xtl, exactly the window)
            tdacc = sb.tile([P, 128], f32)
            nc.vector.memset(tdacc[:], 0.0)
            with tc.tile_pool(name="fw", bufs=4) as fwp:
                for g in range(128):
                    j, r = g // 64, g % 64
                    fw = fwp.tile([P, 1, S], f32)
                    nc.gpsimd.indirect_dma_start(
                        out=fw[:], out_offset=None,
                        in_=inst_d[:],
                        in_offset=bass.IndirectOffsetOnAxis(
                            ap=ridx[:, g:g + 1], axis=1),
                    )
                    tjk = fwp.tile([P, 1, S], f32)
                    nc.vector.scalar_tensor_tensor(
                        out=tjk[:], in0=fw[:],
                        scalar=cg[:, j, 0:1],
                        in1=dm[:, S * j + r:S * j + r + 1, :],
                        op0=Alu.is_equal, op1=Alu.mult,
                        accum_out=tdacc[:, g:g + 1])
            nc.vector.tensor_reduce(out=acc[:, 5:6], in_=tdacc[:, 0:64],
                                    axis=AxX, op=Alu.add)
            nc.vector.tensor_reduce(out=acc[:, 6:7], in_=tdacc[:, 64:128],
                                    axis=AxX, op=Alu.add)

            # --- class loss partials (stable log-softmax at the label)
            M = sb.tile([P, 2], f32)
            for j in range(2):
                nc.vector.tensor_reduce(out=M[:, j:j + 1], in_=sem[:, j],
                                        axis=AxX, op=Alu.max)
            negM = sb.tile([P, 2], f32)
            nc.vector.tensor_scalar(out=negM[:], in0=M[:], scalar1=-1.0,
                                    scalar2=None, op0=Alu.mult)
            sexp = sb.tile([P, 2], f32)
            eb = sb.tile([P, 2, NCLS], f32)
            for j in range(2):
                nc.scalar.activation(out=eb[:, j], in_=sem[:, j], func=Act.Exp,
                                     bias=negM[:, j:j + 1], scale=1.0,
                                     accum_out=sexp[:, j:j + 1])
            lnS = sb.tile([P, 2], f32)
            nc.scalar.activation(out=lnS[:], in_=sexp[:], func=Act.Ln,
                                 bias=0.0, scale=1.0, accum_out=acc[:, 11:12])
            io_i = sb.tile([P, NCLS], i32)
            nc.gpsimd.iota(io_i[:], pattern=[[1, NCLS]], base=0,
                           channel_multiplier=0)
            io_f = sb.tile([P, NCLS], f32)
            nc.vector.tensor_copy(io_f[:], io_i[:])
            xl = sb.tile([P, 2], f32)
            for j in range(2):
                tmp20 = sb.tile([P, NCLS], f32)
                nc.vector.scalar_tensor_tensor(out=tmp20[:], in0=io_f[:],
                                               scalar=cg[:, j, 3:4],
                                               in1=sem[:, j],
                                               op0=Alu.is_equal, op1=Alu.mult,
                                               accum_out=xl[:, j:j + 1])
            v = sb.tile([P, 2], f32)
            nc.vector.tensor_tensor(out=v[:], in0=M[:], in1=lnS[:], op=Alu.add)
            v2 = sb.tile([P, 2], f32)
            nc.vector.tensor_tensor(out=v2[:], in0=v[:], in1=xl[:],
                                    op=Alu.subtract)
            nc.vector.tensor_reduce(out=acc[:, 8:9], in_=v2[:], axis=AxX,
                                    op=Alu.add)

            # --- size loss partials: |true - pred| / (true + eps)
            # true+eps > 0, so |d| * rec == |d * rec| and the abs can ride
            # on the reduce
            tsz = cg[:, :, 1:3]
            d = sb.tile([P, 2, 2], f32)
            nc.vector.tensor_tensor(out=d[:], in0=tsz, in1=szp[:],
                                    op=Alu.subtract)
            den = sb.tile([P, 2, 2], f32)
            nc.vector.tensor_scalar(out=den[:], in0=tsz, scalar1=EPS,
                                    scalar2=None, op0=Alu.add)
            rec = sb.tile([P, 2, 2], f32)
            nc.vector.reciprocal(out=rec[:], in_=den[:])
            q = sb.tile([P, 2, 2], f32)
            nc.vector.tensor_tensor(out=q[:], in0=d[:], in1=rec[:],
                                    op=Alu.mult)
            nc.vector.tensor_reduce(out=acc[:, 7:8], in_=q[:],
                                    axis=mybir.AxisListType.XY, op=Alu.add,
                                    apply_absolute_value=True)

            # --- cross-partition reduction of the 16 accumulator columns
            ones = sb.tile([P, 1], f32)
            nc.vector.memset(ones[:], 1.0)
            psum = ps.tile([16, 1], f32, space="PSUM")
            nc.tensor.matmul(out=psum[:], lhsT=acc[:], rhs=ones[:],
                             start=True, stop=True)
            res = sb.tile([16, 1], f32)
            nc.vector.tensor_copy(res[:], psum[:])
            nc.sync.dma_start(out=out_d[:], in_=res[:, 0])

    nc.compile()
    return nc


def _prepare_generic(inputs):
    heatmap = np.ascontiguousarray(np.asarray(inputs["heatmap"], dtype=np.float32))
    size_pred = np.ascontiguousarray(np.asarray(inputs["size_pred"], dtype=np.float32))
    semantic_pred = np.ascontiguousarray(np.asarray(inputs["semantic_pred"], dtype=np.float32))
    instance_masks = np.ascontiguousarray(np.asarray(inputs["instance_masks"], dtype=np.float32))
    target = np.ascontiguousarray(np.asarray(inputs["target"], dtype=np.float32))
    centers = np.asarray(inputs["centers_bij"]).astype(np.int64)
    boxes = np.asarray(inputs["boxes"]).astype(np.int64)

    batch_lists = []
    for c in range(8):
        sl = slice(KC * c, KC * (c + 1))
        bcl = np.clip(centers[sl, 0], 0, B - 1)
        blist = [c] + [x for x in dict.fromkeys(bcl.tolist()) if x != c]
        batch_lists.append(blist)
    nb = max(len(bl) for bl in batch_lists)

    in_maps = []
    for c in range(8):
        sl = slice(KC * c, KC * (c + 1))
        bcl = np.clip(centers[sl, 0], 0, B - 1)
        ci = np.clip(centers[sl, 1], 0, H - 1)
        cj = np.clip(centers[sl, 2], 0, W - 1)
        blist = list(batch_lists[c])
        blist += [c] * (nb - len(blist))
        lut = np.zeros(B, np.int64)
        seen = {}
        for i, bb in enumerate(blist):
            seen.setdefault(bb, i)
        for bb, i in seen.items():
            lut[bb] = i
        bl = lut[bcl]
        cidx = (bl * PIX + ci * W + cj).astype(np.int32)
        ytl = np.clip(boxes[sl, 1], 0, H - S)
        xtl = np.clip(boxes[sl, 0], 0, W - S)
        # column g = 64*j + r: start of center (2p+j)'s crop row r
        ridx = np.zeros((P, 128), np.int64)
        for g in range(128):
            j, r = g // 64, g % 64
            kk = 2 * np.arange(P) + j
            ridx[:, g] = bl[kk] * PIX + (ytl[kk] + r) * W + xtl[kk]
        ridx = ridx.astype(np.int32)
        in_maps.append({
            "hm": heatmap[c].reshape(P, 512),
            "tgt": np.ascontiguousarray(target[np.array(blist)]).reshape(1, nb * PIX, 7),
            "msk": instance_masks[sl].reshape(P, 2 * S, S),
            "sem": semantic_pred[sl].reshape(P, 2, NCLS),
            "szp": size_pred[sl].reshape(P, 2, 2),
            "cidx": np.ascontiguousarray(cidx.reshape(P, 2)),
            "ridx": np.ascontiguousarray(ridx),
            "inst": np.concatenate([
                np.ascontiguousarray(target[np.array(blist)][:, :, :, 1]).reshape(-1),
                np.zeros(256, np.float32)]).reshape(1, nb * PIX + 256, 1),
        })
    return nb, in_maps


def _combine(parts, fast):
    tot = np.stack([np.asarray(p, dtype=np.float64) for p in parts]).sum(axis=0)
    if fast:
        num_pos = float(K)
        pos_l, neg_l = tot[1], tot[2]
        spd = SP_ALPHA * (tot[3] + tot[4]) + SP_GAMMA * (K * S * S)
        td = tot[5] + tot[6] + tot[7] + tot[8]
        size_s = tot[10]
        cls_s = tot[12] + tot[11] - tot[13] - tot[14]
        loss_shape = (spd - td) / (K * S * S)
    else:
        num_pos, pos_l, neg_l, s_dm, s_sp, td0, td1, size_s, cls_s = tot[:9]
        loss_shape = (s_sp + s_dm - (td0 + td1)) / (K * S * S)
    loss_center = -(pos_l + neg_l) / num_pos
    loss_size = size_s / K
    loss_class = cls_s / K
    return np.asarray(loss_center + loss_size + loss_shape + loss_class,
                      dtype=np.float32)


def kernel(**inputs):
    global LAST_EXEC_NS, LAST_PROFILE
    from concourse import bass_utils
    fast = _is_grid(inputs) and _fast_ok(inputs)
    if fast:
        bf_ok = _bf16_exact(inputs)
        key = ("fast", bf_ok)
        if key not in _CACHE:
            _CACHE[key] = _build_fast(bf_ok)
        nc = _CACHE[key]
        in_maps = _prepare_fast(inputs, bf_ok)
    else:
        nb, in_maps = _prepare_generic(inputs)
        if nb not in _CACHE:
            _CACHE[nb] = _build_generic(nb)
        nc = _CACHE[nb]
    res = bass_utils.run_bass_kernel_spmd(nc, in_maps, list(range(8)), trace=TRACE)
    LAST_EXEC_NS = res.exec_time_ns
    LAST_PROFILE = res.profile_json
    return _combine([r["out"] for r in res.results], fast)
